# revision 32
# baseline (speedup 1.0000x reference)
"""CodeShell attention block (B=1, S=2048, 32 Q heads / 8 KV heads GQA,
head_dim=128) as a Trainium2 Bass/Tile kernel, tensor-parallel over heads
across 8 NeuronCores.

Sharding: core c owns Q heads 4c..4c+3 and KV head c (Wqkv column shard
[4096,768], Wproj row shard [512,4096]). Each core computes a partial
yT = Wp_c.T @ attn_c in transposed layout; the all-reduce (sum of the 8
partials) + final bias happen on the host after gather.

Device strategy (per core, everything "transposed"):
  Phase 1 (qkvT = Wc.T @ hT + b) runs in error-compensated fp8 with
  DoubleRow perf mode: X and W are split on the host into fp8e4 hi/lo
  parts at a shared scale (X ~ Xh+Xl, W ~ Wh+Wl), and the device chains
  Xh@Wh + Xh@Wl + Xl@Wh as 48 DoubleRow matmuls (2 k-tiles per pass at
  0.5 cycles/row) per output tile = 0.75x the bf16 PE time with ~bf16
  accuracy (the dropped Xl@Wl term is ~0.1% of sigma).  The fp32 PSUM
  result is descaled+biased by the drain activation, written bf16.
  RoPE in bf16, in place, on the 4 q tiles and the k tile (all-SBUF
  2-byte DVE ops run in 4x perf mode).
  scoresT [k,q] blocks (bf16 matmul) -> exp on ACT (no max-subtraction
  needed: |scores| < ~10 for this distribution) -> causal handled at
  block granularity, with partial-width matmuls below the diagonal and a
  0/1 mask multiply on diagonal blocks -> OT[d,q] += V_blk.T @ PT, plus
  a ones-column matmul accumulating softmax denominators in the same
  pass -> normalize via GPSIMD partition_broadcast of 1/s (no PE/ACT
  involvement) -> yT = Wp.T @ OT (bf16) streamed out as fp16 partials,
  PSUM drains alternating between ACT and DVE.
"""
import numpy as np
import ml_dtypes

import concourse.bass as bass
import concourse.mybir as mybir
import concourse.tile as tile
from concourse.bass_utils import run_bass_kernel_spmd
from concourse.vector_clock import ScopedClock, VectorClock

BF16NP = ml_dtypes.bfloat16
E4NP = ml_dtypes.float8_e4m3


class PatchedTileContext(tile.TileContext):
    """TileContext whose kernel-tail drain carries at most one sem wait per
    instruction: the public walrus here rejects a Drain with more than one
    sync wait. Waits are front-loaded onto per-processor NOPs on the SP
    queue (which executes in order), leaving the drain itself waitless."""

    def _drain_and_barrier(self, tick_clock, wait_clock):
        nc = self.nc
        gc = tick_clock.global_clock
        n = len(gc)
        for proc in range(n):
            tick = gc[proc]
            if tick <= 0:
                continue
            vec = [0] * n
            vec[proc] = tick
            nop = nc.sync.nop()
            wait_clock.add_sem_waits(nop.ins, ScopedClock({None: VectorClock(vec)}))
        nc.sync.drain()
        nc.all_engine_barrier()
        assert self.sems is not None
        popped = nc._tile_sem_poison_stack.pop()
        assert popped is self._sem_poison
        nc.clear_and_free_semaphores(list(self.sems.allocated().values()))
        nc.all_engine_barrier()


F32 = mybir.dt.float32
BF16 = mybir.dt.bfloat16
F16 = mybir.dt.float16
FP8 = mybir.dt.float8e4
AF = mybir.ActivationFunctionType
DR = mybir.MatmulPerfMode.DoubleRow


def _split_multi_waits(nc):
    """The public walrus in this container encodes at most one sync wait and
    one sync update per instruction (one TPB EVENTS struct). Tile's sem
    assignment freely emits several. Split them: extra waits move onto
    same-engine NOPs inserted immediately before the instruction (the
    engine's sequencer processes waits in program order, so semantics are
    identical); extra updates move onto NOPs immediately after."""
    fn = nc.m.functions[0]
    spawned = set()

    def fresh_nop(engine, on_wait, on_update):
        nbi = nc.engines[engine].nop()
        ninst = nbi.ins
        spawned.add(id(ninst))
        ninst.sync_info = mybir.SyncInfo(on_wait=on_wait, on_update=on_update)
        return ninst

    for bb in fn.blocks:
        newlist = []
        for inst in list(bb.instructions):
            if id(inst) in spawned:
                continue  # already placed next to its parent instruction
            si = inst.sync_info
            waits = list(si.on_wait) if si and si.on_wait else []
            updates = list(si.on_update) if si and si.on_update else []
            pre, post = [], []
            if len(waits) > 1:
                for w in waits[:-1]:
                    pre.append(fresh_nop(inst.engine, [w], []))
                waits = [waits[-1]]
            if len(updates) > 1:
                for u in updates[1:]:
                    post.append(fresh_nop(inst.engine, [], [u]))
                updates = [updates[0]]
            if pre or post:
                inst.sync_info = mybir.SyncInfo(on_wait=waits, on_update=updates)
            newlist.extend(pre)
            newlist.append(inst)
            newlist.extend(post)
        bb.instructions[:] = newlist
    # strip the spawned nops from wherever nc appended them originally
    for bb in fn.blocks:
        seen = set()
        kept = []
        for inst in bb.instructions:
            if id(inst) in spawned:
                if id(inst) in seen:
                    continue
                seen.add(id(inst))
            kept.append(inst)
        bb.instructions[:] = kept


H, S, NH, NKV, D = 4096, 2048, 32, 8, 128
HALF = D // 2
NCORES = 8
QH = NH // NCORES      # 4 q heads per core
MC = 4                 # m-chunks of 512 positions
NT = 6                 # qkvT row tiles per core (4 q heads + k + v)
KK = H // 128          # 32 contraction tiles
JP = KK // 2           # 16 DoubleRow k-tile pairs
NG = 4                 # ht sub-tiles per chunk (4 pairs each)
QC = 4                 # 512-wide q chunks
NJ = S // 128          # 16 key blocks
HT_BUFS = 8            # [128, 8, 512] bf16 sub-tiles; 8 = two full chunks so chunk k+1 prefetches while k computes
ROPE_THETA = 10000.0
SCALE = 0.08838834764831845  # 1/sqrt(head_dim), folded into the Exp activation
SX = 32.0              # fp8 scale for hidden states
SWQ = 1024.0           # fp8 scale for Wqkv
DESCALE = 1.0 / (SX * SWQ)


def _emit_body(nc, tc, aps, upto='full'):
    ht, wq, bq, wp, cost, sint, mask, ident, yt = aps

    cpool = tc.alloc_tile_pool(name="const", bufs=1)
    bq_sb = cpool.tile([128, NT], F32, tag="bq", name="bq_sb")
    nc.sync.dma_start(bq_sb, bq)
    # all-ones [128,128] stationary: the denominator matmul then produces
    # the k-sum broadcast to every output partition at full PE tile config
    # (a [1,512]-out matmul runs ~4x slower and needed a separate broadcast)
    ones_sq = cpool.tile([128, 128], BF16, tag="onesq", name="ones_sq")
    nc.vector.memset(ones_sq, 1.0)

    # qkvT row tiles (bf16): 4 q heads + k (roped in place) + v
    qkvp = tc.alloc_tile_pool(name="qkvT", bufs=1)
    qkvT = [qkvp.tile([128, S], BF16, tag=f"qkvT{nt}", name=f"qkvT{nt}")
            for nt in range(NT - 1)]
    vb = qkvp.tile([128, S], BF16, tag="vb", name="vb")

    # rope/vnat/OT pools are allocated BEFORE the phase-1 scoped pools so
    # their SBUF ranges don't alias the phase-1 pools — address reuse would
    # add anti-deps serializing RoPE/attention behind the whole of phase 1.
    ropep = tc.alloc_tile_pool(name="rope", bufs=1)
    vpool = tc.alloc_tile_pool(name="vnat", bufs=1)
    vnat = vpool.tile([128, S], BF16, tag="vnat", name="vnat")
    otp = tc.alloc_tile_pool(name="OT", bufs=1)
    OT = [otp.tile([128, S], BF16, tag=f"OT{hh}", name=f"OT{hh}")
          for hh in range(QH)]

    # k and v row-tiles first so RoPE(k) / V-transpose / head-0 attention can
    # start while the last chunk's q tiles are still on the PE.
    NT_ORDER = [QH, QH + 1, 0, 1, 2, 3]

    # ---- Phase 1: qkvT = Wc.T @ hT + b  (bf16) ----
    with tc.tile_pool(name="htp", bufs=HT_BUFS) as htp, \
         tc.tile_pool(name="wst", bufs=3) as wsp, \
         tc.tile_pool(name="qkps", bufs=3, space="PSUM") as qkp:
        for mc in range(MC):
            # first compute pass needs the W strip before most ht tiles
            ws0 = wsp.tile([128, 4096], BF16, tag="w", name="ws")
            nc.gpsimd.dma_start(ws0, wq[NT_ORDER[0]])
            hts = []
            for g in range(NG):
                t = htp.tile([128, 8, 512], BF16, tag="ht", name="htt")
                nc.sync.dma_start(t, ht[mc, g])
                hts.append(t)
            if mc == 1:
                # rope/attention constants: needed only from phase 2 on, so
                # emit them mid-phase-1 (DMA queue priority follows program
                # order), spread over two chunks to limit contention
                cos_sb = cpool.tile([128, S], BF16, tag="cost", name="cos_sb")
                nc.sync.dma_start(cos_sb, cost)
                sin_sb = cpool.tile([128, S], BF16, tag="sint", name="sin_sb")
                nc.sync.dma_start(sin_sb, sint)
            if mc == 2:
                mask_sb = cpool.tile([128, 128], BF16, tag="mask", name="mask_sb")
                nc.sync.dma_start(mask_sb, mask)
                id_sb = cpool.tile([128, 128], BF16, tag="ident", name="id_sb")
                nc.sync.dma_start(id_sb, ident)
            for ni, nt in enumerate(NT_ORDER):
                if ni == 0:
                    ws = ws0
                else:
                    ws = wsp.tile([128, 4096], BF16, tag="w", name="ws")
                    nc.gpsimd.dma_start(ws, wq[nt])
                ps = qkp.tile([128, 512], F32, tag="ps", name="qk_ps")
                for k in range(KK):
                    nc.tensor.matmul(ps, ws[:, k * 128:(k + 1) * 128],
                                     hts[k // 8][:, k % 8, :],
                                     start=(k == 0), stop=(k == KK - 1))
                dst = vb if nt == NT - 1 else qkvT[nt]
                nc.scalar.activation(dst[:, mc * 512:(mc + 1) * 512], ps,
                                     AF.Identity, bias=bq_sb[:, nt:nt + 1])

    # ---- Phase 2a: RoPE (bf16, in place) + V transpose to natural layout ----
    # All compute operands share partition ranges (walrus requires it);
    # the cross-half moves go through SBUF->SBUF DMA. sh holds the shifted
    # halves: sh[0:64] = x[64:128], sh[64:128] = x[0:64].
    def rope(nt, part=None):
        # part=None: whole tile.  part=(i, n): emit column chunk i of n.
        x = qkvT[nt]
        if part is None or part[0] == 0:
            sh = ropep.tile([128, S], BF16, tag="sh", name="rope_sh")
            nc.sync.dma_start(sh[0:64], x[64:128])
            nc.sync.dma_start(sh[64:128], x[0:64])
            m1 = ropep.tile([128, S], BF16, tag="m1", name="rope_m1")
            rope.cur = (sh, m1)
        sh, m1 = rope.cur
        if part is None:
            lo, hi = 0, S
        else:
            w = S // part[1]
            lo, hi = part[0] * w, (part[0] + 1) * w
        nc.vector.tensor_mul(m1[0:64, lo:hi], x[0:64, lo:hi], cos_sb[0:64, lo:hi])
        nc.vector.tensor_mul(m1[64:128, lo:hi], x[64:128, lo:hi], cos_sb[64:128, lo:hi])
        nc.vector.tensor_mul(sh[0:64, lo:hi], sh[0:64, lo:hi], sin_sb[0:64, lo:hi])
        nc.vector.tensor_mul(sh[64:128, lo:hi], sh[64:128, lo:hi], sin_sb[64:128, lo:hi])
        nc.vector.tensor_sub(x[0:64, lo:hi], m1[0:64, lo:hi], sh[0:64, lo:hi])
        nc.vector.tensor_add(x[64:128, lo:hi], m1[64:128, lo:hi], sh[64:128, lo:hi])

    rope(QH)  # k tile first so attention can start early
    rope(0)
    with tc.tile_pool(name="vtps", bufs=2, space="PSUM") as vtp:
        for j in range(NJ):
            tp = vtp.tile([128, 128], BF16, tag="vt", name="vt_ps")
            nc.tensor.transpose(tp, vb[:, j * 128:(j + 1) * 128], id_sb)
            nc.vector.tensor_copy(vnat[:, j * 128:(j + 1) * 128], tp)
    # prefetch the first Wproj strips so phase 4 starts without a DMA stall
    wpp = tc.alloc_tile_pool(name="wpp", bufs=6)
    wps_pre = {}

    def fetch_wp(nt):
        w = wpp.tile([128, 512], BF16, tag="wp", name="wp_sb")
        nc.sync.dma_start(w, wp[nt])
        wps_pre[nt] = w

    for nt in range(5):
        fetch_wp(nt)

    # ---- Phase 2b/3: per-head RoPE + attention ----
    kT = qkvT[QH]
    heads = range(QH) if upto in ('full', 'attn') else range(0)
    with tc.tile_pool(name="stps", bufs=4, space="PSUM") as stp, \
         tc.tile_pool(name="otps", bufs=2, space="PSUM") as otps, \
         tc.tile_pool(name="sps", bufs=2, space="PSUM") as sps, \
         tc.tile_pool(name="ptp", bufs=6) as ptp, \
         tc.tile_pool(name="rbp", bufs=2) as rbsb:
        for hh in heads:
            qT = qkvT[hh]
            for qc in range(QC):
                ot_ps = otps.tile([128, 512], F32, tag="ot", name="ot_ps")
                s_ps = sps.tile([128, 512], F32, tag="s", name="s_ps")
                nj = 4 * qc + 4
                # Software pipeline, depth K: emit score_j K iterations ahead
                # of pv_j/ones_j. Engine sequencers are strictly in-order, so
                # pv_j's wait on exp_j would otherwise park the PE queue and
                # serialize the whole loop at exp latency per block.
                KD = min(4, nj)
                pts = {}

                def score(j):
                    # diagonal blocks: columns below the block diagonal are
                    # non-causal for every k-row in the block — skip them
                    # entirely (partial-width matmuls); only [off,off+128)
                    # needs the triangular mask.
                    diag = j >= 4 * qc
                    off = (j - 4 * qc) * 128 if diag else 0
                    st_ps = stp.tile([128, 512], F32, tag="st", name="st_ps")
                    nc.tensor.matmul(st_ps[:, off:], kT[:, j * 128:(j + 1) * 128],
                                     qT[:, qc * 512 + off:(qc + 1) * 512],
                                     start=True, stop=True)
                    pt = ptp.tile([128, 512], BF16, tag="pt", name="pt")
                    nc.scalar.activation(pt[:, off:], st_ps[:, off:], AF.Exp, scale=SCALE)
                    if diag:
                        nc.vector.tensor_mul(pt[:, off:off + 128],
                                             pt[:, off:off + 128], mask_sb)
                    pts[j] = (pt, off)

                def accum(j):
                    pt, off = pts.pop(j)
                    nc.tensor.matmul(ot_ps[:, off:], vnat[:, j * 128:(j + 1) * 128],
                                     pt[:, off:],
                                     start=(j == 0), stop=(j == nj - 1))
                    nc.tensor.matmul(s_ps[:, off:], ones_sq, pt[:, off:],
                                     start=(j == 0), stop=(j == nj - 1))

                for j in range(nj + KD):
                    if j < nj:
                        score(j)
                    if j >= KD:
                        accum(j - KD)
                rb = rbsb.tile([128, 512], BF16, tag="rbs", name="rb")
                nc.vector.reciprocal(rb, s_ps)
                nc.vector.tensor_mul(OT[hh][:, qc * 512:(qc + 1) * 512], ot_ps, rb)
                if hh + 1 < QH:
                    # next head's in-place rotation trickles out under this
                    # head's attention, one column chunk per qc
                    rope(hh + 1, part=(qc, QC))

    # ---- Phase 4: yT = Wp.T @ OT, streamed out as fp16 partials; PSUM
    # drains alternate ACT/DVE; one batched DMA per 128-row tile ----
    with tc.tile_pool(name="ysb", bufs=6) as ysp, \
         tc.tile_pool(name="yps", bufs=6, space="PSUM") as ypp:
        for nt in (range(32) if upto in ('full', 'p4') else range(0)):
            if nt + 5 < 32 and nt + 5 not in wps_pre:
                fetch_wp(nt + 5)
            if nt in wps_pre:
                wps = wps_pre.pop(nt)
            else:
                wps = wpp.tile([128, 512], BF16, tag="wp", name="wp_sb")
                nc.sync.dma_start(wps, wp[nt])
            ysb = ysp.tile([128, S], F16, tag="ysb", name="y_sb")
            for qc in range(QC):
                yps = ypp.tile([128, 512], F32, tag="y", name="y_ps")
                for kb in range(QH):
                    nc.tensor.matmul(yps, wps[:, kb * 128:(kb + 1) * 128],
                                     OT[kb][:, qc * 512:(qc + 1) * 512],
                                     start=(kb == 0), stop=(kb == QH - 1))
                if qc % 2 == 0:
                    nc.scalar.copy(ysb[:, qc * 512:(qc + 1) * 512], yps)
                else:
                    nc.vector.tensor_copy(ysb[:, qc * 512:(qc + 1) * 512], yps)
            nc.scalar.dma_start(yt[nt], ysb)

    for p in (wpp, otp, vpool, ropep, qkvp, cpool):
        p.release()


def build_program(reps=1, upto='full'):
    nc = bass.Bass("TRN2", target_bir_lowering=False, debug=False)
    ht = nc.dram_tensor("ht", [MC, NG, 128, 8, 512], BF16, kind="ExternalInput").ap()
    wq = nc.dram_tensor("wq", [NT, 128, 4096], BF16, kind="ExternalInput").ap()
    bq = nc.dram_tensor("bq", [128, NT], F32, kind="ExternalInput").ap()
    wp = nc.dram_tensor("wp", [32, 128, 512], BF16, kind="ExternalInput").ap()
    cost = nc.dram_tensor("cost", [128, S], BF16, kind="ExternalInput").ap()
    sint = nc.dram_tensor("sint", [128, S], BF16, kind="ExternalInput").ap()
    mask = nc.dram_tensor("mask", [128, 128], BF16, kind="ExternalInput").ap()
    ident = nc.dram_tensor("ident", [128, 128], BF16, kind="ExternalInput").ap()
    yt = nc.dram_tensor("yt", [32, 128, S], F16, kind="ExternalOutput").ap()
    aps = (ht, wq, bq, wp, cost, sint, mask, ident, yt)

    ctx_lp = nc.allow_low_precision(reason="compensated fp8 / bf16 matmuls; fp32 PSUM accumulation")
    ctx_lp.__enter__()
    with PatchedTileContext(nc) as tc:
        for _rep in range(reps):
            _emit_body(nc, tc, aps, upto=upto)
    ctx_lp.__exit__(None, None, None)
    _split_multi_waits(nc)
    return nc


def host_prep(positions, hidden_states, Wqkv, bqkv, Wproj):
    pos = np.asarray(positions).reshape(S)
    h = np.asarray(hidden_states, dtype=np.float32).reshape(S, H)
    Wqkv = np.asarray(Wqkv, dtype=np.float32)
    bqkv = np.asarray(bqkv, dtype=np.float32)
    Wproj = np.asarray(Wproj, dtype=np.float32)

    # hT as [MC, NG, 128, 8, 512] bf16 (8 k-tiles per DMA sub-tile)
    ht_t = np.ascontiguousarray(
        h.reshape(MC, 512, NG, 8, 128).transpose(0, 2, 4, 3, 1)).astype(BF16NP)

    inv_freq = (np.float32(1.0) / (np.float32(ROPE_THETA) **
                (np.arange(HALF, dtype=np.float32) / np.float32(HALF)))).astype(np.float32)
    ang = pos.astype(np.float32)[:, None] * inv_freq[None, :]
    cos = np.cos(ang).astype(np.float32).T     # [64, S]
    sin = np.sin(ang).astype(np.float32).T
    cost = np.ascontiguousarray(np.concatenate([cos, cos], axis=0)).astype(BF16NP)
    sint = np.ascontiguousarray(np.concatenate([sin, sin], axis=0)).astype(BF16NP)

    dk = np.arange(128)[:, None]
    dq = np.arange(128)[None, :]
    mask = np.ascontiguousarray((dq >= dk).astype(np.float32)).astype(BF16NP)
    ident = np.eye(128, dtype=np.float32).astype(BF16NP)

    shared = {"ht": ht_t, "cost": cost, "sint": sint,
              "mask": mask, "ident": ident}
    per_core = []
    for c in range(NCORES):
        Wc = np.concatenate([
            Wqkv[:, 512 * c: 512 * (c + 1)],
            Wqkv[:, H + 128 * c: H + 128 * (c + 1)],
            Wqkv[:, H + 1024 + 128 * c: H + 1024 + 128 * (c + 1)],
        ], axis=1)
        bc = np.concatenate([
            bqkv[512 * c: 512 * (c + 1)],
            bqkv[H + 128 * c: H + 128 * (c + 1)],
            bqkv[H + 1024 + 128 * c: H + 1024 + 128 * (c + 1)],
        ])
        wq_t = np.ascontiguousarray(
            Wc.reshape(KK, 128, NT, 128).transpose(2, 1, 0, 3).reshape(NT, 128, 4096)).astype(BF16NP)
        bq_t = np.ascontiguousarray(bc.reshape(NT, 128).T)
        Wp_c = Wproj[512 * c: 512 * (c + 1), :]
        wp_t = np.ascontiguousarray(
            Wp_c.reshape(QH, 128, 32, 128).transpose(2, 1, 0, 3).reshape(32, 128, 512)).astype(BF16NP)
        per_core.append({"wq": wq_t, "bq": bq_t, "wp": wp_t})
    return shared, per_core


_NC = None


def _get_nc():
    global _NC
    if _NC is None:
        _NC = build_program()
    return _NC


def kernel(positions, hidden_states, Wqkv, bqkv, Wproj, bproj):
    shared, per_core = host_prep(positions, hidden_states, Wqkv, bqkv, Wproj)
    nc = _get_nc()
    in_maps = [dict(shared, **per_core[c]) for c in range(NCORES)]
    res = run_bass_kernel_spmd(nc, in_maps, core_ids=list(range(NCORES)))
    acc = np.zeros((H, S), np.float32)
    for c in range(NCORES):
        acc += res.results[c]["yt"].astype(np.float32).reshape(H, S)
    y = acc.T + np.asarray(bproj, dtype=np.float32)[None, :]
    return y.reshape(1, S, H).astype(np.float32)


# revision 33
# speedup vs baseline: 1.0101x; 1.0101x over previous
"""CodeShell attention block (B=1, S=2048, 32 Q heads / 8 KV heads GQA,
head_dim=128) as a Trainium2 Bass/Tile kernel, tensor-parallel over heads
across 8 NeuronCores.

Sharding: core c owns Q heads 4c..4c+3 and KV head c (Wqkv column shard
[4096,768], Wproj row shard [512,4096]). Each core computes a partial
yT = Wp_c.T @ attn_c in transposed layout; the all-reduce (sum of the 8
partials) + final bias happen on the host after gather.

Device strategy (per core, everything "transposed"):
  Phase 1 (qkvT = Wc.T @ hT + b) runs in error-compensated fp8 with
  DoubleRow perf mode: X and W are split on the host into fp8e4 hi/lo
  parts at a shared scale (X ~ Xh+Xl, W ~ Wh+Wl), and the device chains
  Xh@Wh + Xh@Wl + Xl@Wh as 48 DoubleRow matmuls (2 k-tiles per pass at
  0.5 cycles/row) per output tile = 0.75x the bf16 PE time with ~bf16
  accuracy (the dropped Xl@Wl term is ~0.1% of sigma).  The fp32 PSUM
  result is descaled+biased by the drain activation, written bf16.
  RoPE in bf16, in place, on the 4 q tiles and the k tile (all-SBUF
  2-byte DVE ops run in 4x perf mode).
  scoresT [k,q] blocks (bf16 matmul) -> exp on ACT (no max-subtraction
  needed: |scores| < ~10 for this distribution) -> causal handled at
  block granularity, with partial-width matmuls below the diagonal and a
  0/1 mask multiply on diagonal blocks -> OT[d,q] += V_blk.T @ PT, plus
  a ones-column matmul accumulating softmax denominators in the same
  pass -> normalize via GPSIMD partition_broadcast of 1/s (no PE/ACT
  involvement) -> yT = Wp.T @ OT (bf16) streamed out as fp16 partials,
  PSUM drains alternating between ACT and DVE.
"""
import numpy as np
import ml_dtypes

import concourse.bass as bass
import concourse.mybir as mybir
import concourse.tile as tile
from concourse.bass_utils import run_bass_kernel_spmd
from concourse.vector_clock import ScopedClock, VectorClock

BF16NP = ml_dtypes.bfloat16
E4NP = ml_dtypes.float8_e4m3


class PatchedTileContext(tile.TileContext):
    """TileContext whose kernel-tail drain carries at most one sem wait per
    instruction: the public walrus here rejects a Drain with more than one
    sync wait. Waits are front-loaded onto per-processor NOPs on the SP
    queue (which executes in order), leaving the drain itself waitless."""

    def _drain_and_barrier(self, tick_clock, wait_clock):
        nc = self.nc
        gc = tick_clock.global_clock
        n = len(gc)
        for proc in range(n):
            tick = gc[proc]
            if tick <= 0:
                continue
            vec = [0] * n
            vec[proc] = tick
            nop = nc.sync.nop()
            wait_clock.add_sem_waits(nop.ins, ScopedClock({None: VectorClock(vec)}))
        nc.sync.drain()
        nc.all_engine_barrier()
        assert self.sems is not None
        popped = nc._tile_sem_poison_stack.pop()
        assert popped is self._sem_poison
        nc.clear_and_free_semaphores(list(self.sems.allocated().values()))
        nc.all_engine_barrier()


F32 = mybir.dt.float32
BF16 = mybir.dt.bfloat16
F16 = mybir.dt.float16
FP8 = mybir.dt.float8e4
AF = mybir.ActivationFunctionType
DR = mybir.MatmulPerfMode.DoubleRow


def _split_multi_waits(nc):
    """The public walrus in this container encodes at most one sync wait and
    one sync update per instruction (one TPB EVENTS struct). Tile's sem
    assignment freely emits several. Split them: extra waits move onto
    same-engine NOPs inserted immediately before the instruction (the
    engine's sequencer processes waits in program order, so semantics are
    identical); extra updates move onto NOPs immediately after."""
    fn = nc.m.functions[0]
    spawned = set()

    def fresh_nop(engine, on_wait, on_update):
        nbi = nc.engines[engine].nop()
        ninst = nbi.ins
        spawned.add(id(ninst))
        ninst.sync_info = mybir.SyncInfo(on_wait=on_wait, on_update=on_update)
        return ninst

    for bb in fn.blocks:
        newlist = []
        for inst in list(bb.instructions):
            if id(inst) in spawned:
                continue  # already placed next to its parent instruction
            si = inst.sync_info
            waits = list(si.on_wait) if si and si.on_wait else []
            updates = list(si.on_update) if si and si.on_update else []
            pre, post = [], []
            if len(waits) > 1:
                for w in waits[:-1]:
                    pre.append(fresh_nop(inst.engine, [w], []))
                waits = [waits[-1]]
            if len(updates) > 1:
                for u in updates[1:]:
                    post.append(fresh_nop(inst.engine, [], [u]))
                updates = [updates[0]]
            if pre or post:
                inst.sync_info = mybir.SyncInfo(on_wait=waits, on_update=updates)
            newlist.extend(pre)
            newlist.append(inst)
            newlist.extend(post)
        bb.instructions[:] = newlist
    # strip the spawned nops from wherever nc appended them originally
    for bb in fn.blocks:
        seen = set()
        kept = []
        for inst in bb.instructions:
            if id(inst) in spawned:
                if id(inst) in seen:
                    continue
                seen.add(id(inst))
            kept.append(inst)
        bb.instructions[:] = kept


H, S, NH, NKV, D = 4096, 2048, 32, 8, 128
HALF = D // 2
NCORES = 8
QH = NH // NCORES      # 4 q heads per core
MC = 4                 # m-chunks of 512 positions
NT = 6                 # qkvT row tiles per core (4 q heads + k + v)
KK = H // 128          # 32 contraction tiles
JP = KK // 2           # 16 DoubleRow k-tile pairs
NG = 4                 # ht sub-tiles per chunk (4 pairs each)
QC = 4                 # 512-wide q chunks
NJ = S // 128          # 16 key blocks
HT_BUFS = 8            # [128, 8, 512] bf16 sub-tiles; 8 = two full chunks so chunk k+1 prefetches while k computes
ROPE_THETA = 10000.0
SCALE = 0.08838834764831845  # 1/sqrt(head_dim), folded into the Exp activation
SX = 32.0              # fp8 scale for hidden states
SWQ = 1024.0           # fp8 scale for Wqkv
DESCALE = 1.0 / (SX * SWQ)


def _emit_body(nc, tc, aps, upto='full'):
    ht, wq, bq, wp, cost, sint, mask, ident, yt = aps

    cpool = tc.alloc_tile_pool(name="const", bufs=1)
    bq_sb = cpool.tile([128, NT], F32, tag="bq", name="bq_sb")
    nc.sync.dma_start(bq_sb, bq)
    # all-ones [128,128] stationary: the denominator matmul then produces
    # the k-sum broadcast to every output partition at full PE tile config
    # (a [1,512]-out matmul runs ~4x slower and needed a separate broadcast)
    ones_sq = cpool.tile([128, 128], BF16, tag="onesq", name="ones_sq")
    nc.vector.memset(ones_sq, 1.0)

    # qkvT row tiles (bf16): 4 q heads + k (roped in place) + v
    qkvp = tc.alloc_tile_pool(name="qkvT", bufs=1)
    qkvT = [qkvp.tile([128, S], BF16, tag=f"qkvT{nt}", name=f"qkvT{nt}")
            for nt in range(NT - 1)]
    vb = qkvp.tile([128, S], BF16, tag="vb", name="vb")

    # rope/vnat/OT pools are allocated BEFORE the phase-1 scoped pools so
    # their SBUF ranges don't alias the phase-1 pools — address reuse would
    # add anti-deps serializing RoPE/attention behind the whole of phase 1.
    ropep = tc.alloc_tile_pool(name="rope", bufs=1)
    vpool = tc.alloc_tile_pool(name="vnat", bufs=1)
    vnat = vpool.tile([128, S], BF16, tag="vnat", name="vnat")
    otp = tc.alloc_tile_pool(name="OT", bufs=1)
    OT = [otp.tile([128, S], BF16, tag=f"OT{hh}", name=f"OT{hh}")
          for hh in range(QH)]

    # k and v row-tiles first so RoPE(k) / V-transpose / head-0 attention can
    # start while the last chunk's q tiles are still on the PE.
    NT_ORDER = [QH, QH + 1, 0, 1, 2, 3]

    # ---- Phase 1: qkvT = Wc.T @ hT + b  (bf16) ----
    with tc.tile_pool(name="htp", bufs=HT_BUFS) as htp, \
         tc.tile_pool(name="wst", bufs=3) as wsp, \
         tc.tile_pool(name="qkps", bufs=3, space="PSUM") as qkp:
        for mc in range(MC):
            # first compute pass needs the W strip before most ht tiles
            ws0 = wsp.tile([128, 4096], BF16, tag="w", name="ws")
            nc.gpsimd.dma_start(ws0, wq[NT_ORDER[0]])
            hts = []
            for g in range(NG):
                t = htp.tile([128, 8, 512], BF16, tag="ht", name="htt")
                nc.sync.dma_start(t, ht[mc, g])
                hts.append(t)
            if mc == 1:
                # rope/attention constants: needed only from phase 2 on, so
                # emit them mid-phase-1 (DMA queue priority follows program
                # order), spread over two chunks to limit contention
                cos_sb = cpool.tile([128, S], BF16, tag="cost", name="cos_sb")
                nc.sync.dma_start(cos_sb, cost)
                sin_sb = cpool.tile([128, S], BF16, tag="sint", name="sin_sb")
                nc.sync.dma_start(sin_sb, sint)
            if mc == 2:
                mask_sb = cpool.tile([128, 128], BF16, tag="mask", name="mask_sb")
                nc.sync.dma_start(mask_sb, mask)
                id_sb = cpool.tile([128, 128], BF16, tag="ident", name="id_sb")
                nc.sync.dma_start(id_sb, ident)
            for ni, nt in enumerate(NT_ORDER):
                if ni == 0:
                    ws = ws0
                else:
                    ws = wsp.tile([128, 4096], BF16, tag="w", name="ws")
                    nc.gpsimd.dma_start(ws, wq[nt])
                ps = qkp.tile([128, 512], F32, tag="ps", name="qk_ps")
                for k in range(KK):
                    nc.tensor.matmul(ps, ws[:, k * 128:(k + 1) * 128],
                                     hts[k // 8][:, k % 8, :],
                                     start=(k == 0), stop=(k == KK - 1))
                dst = vb if nt == NT - 1 else qkvT[nt]
                nc.scalar.activation(dst[:, mc * 512:(mc + 1) * 512], ps,
                                     AF.Identity, bias=bq_sb[:, nt:nt + 1])

    # ---- Phase 2a: RoPE (bf16, in place) + V transpose to natural layout ----
    # All compute operands share partition ranges (walrus requires it);
    # the cross-half moves go through SBUF->SBUF DMA. sh holds the shifted
    # halves: sh[0:64] = x[64:128], sh[64:128] = x[0:64].
    def rope(nt, part=None):
        # part=None: whole tile.  part=(i, n): emit column chunk i of n.
        x = qkvT[nt]
        if part is None or part[0] == 0:
            sh = ropep.tile([128, S], BF16, tag="sh", name="rope_sh")
            nc.sync.dma_start(sh[0:64], x[64:128])
            nc.sync.dma_start(sh[64:128], x[0:64])
            m1 = ropep.tile([128, S], BF16, tag="m1", name="rope_m1")
            rope.cur = (sh, m1)
        sh, m1 = rope.cur
        if part is None:
            lo, hi = 0, S
        else:
            w = S // part[1]
            lo, hi = part[0] * w, (part[0] + 1) * w
        nc.vector.tensor_mul(m1[0:64, lo:hi], x[0:64, lo:hi], cos_sb[0:64, lo:hi])
        nc.vector.tensor_mul(m1[64:128, lo:hi], x[64:128, lo:hi], cos_sb[64:128, lo:hi])
        nc.vector.tensor_mul(sh[0:64, lo:hi], sh[0:64, lo:hi], sin_sb[0:64, lo:hi])
        nc.vector.tensor_mul(sh[64:128, lo:hi], sh[64:128, lo:hi], sin_sb[64:128, lo:hi])
        nc.vector.tensor_sub(x[0:64, lo:hi], m1[0:64, lo:hi], sh[0:64, lo:hi])
        nc.vector.tensor_add(x[64:128, lo:hi], m1[64:128, lo:hi], sh[64:128, lo:hi])

    rope(QH)  # k tile first so attention can start early
    rope(0)
    with tc.tile_pool(name="vtps", bufs=2, space="PSUM") as vtp:
        for j in range(NJ):
            tp = vtp.tile([128, 128], BF16, tag="vt", name="vt_ps")
            nc.tensor.transpose(tp, vb[:, j * 128:(j + 1) * 128], id_sb)
            nc.vector.tensor_copy(vnat[:, j * 128:(j + 1) * 128], tp)
    # prefetch the first Wproj strips so phase 4 starts without a DMA stall
    wpp = tc.alloc_tile_pool(name="wpp", bufs=32)
    wps_pre = {}

    def fetch_wp(nt):
        w = wpp.tile([128, 512], BF16, tag="wp", name="wp_sb")
        nc.sync.dma_start(w, wp[nt])
        wps_pre[nt] = w

    for nt in range(32):
        fetch_wp(nt)

    # ---- Phase 2b/3: per-head RoPE + attention ----
    kT = qkvT[QH]
    heads = range(QH) if upto in ('full', 'attn') else range(0)
    with tc.tile_pool(name="stps", bufs=3, space="PSUM") as stp, \
         tc.tile_pool(name="otps", bufs=2, space="PSUM") as otps, \
         tc.tile_pool(name="sps", bufs=2, space="PSUM") as sps, \
         tc.tile_pool(name="ptp", bufs=6) as ptp, \
         tc.tile_pool(name="rbp", bufs=2) as rbsb:
        for hh in heads:
            qT = qkvT[hh]
            for qc in range(QC):
                ot_ps = otps.tile([128, 512], F32, tag="ot", name="ot_ps")
                s_ps = sps.tile([128, 512], F32, tag="s", name="s_ps")
                nj = 4 * qc + 4
                # Software pipeline, depth K: emit score_j K iterations ahead
                # of pv_j/ones_j. Engine sequencers are strictly in-order, so
                # pv_j's wait on exp_j would otherwise park the PE queue and
                # serialize the whole loop at exp latency per block.
                KD = min(3, nj)
                pts = {}

                def score(j):
                    # diagonal blocks: columns below the block diagonal are
                    # non-causal for every k-row in the block — skip them
                    # entirely (partial-width matmuls); only [off,off+128)
                    # needs the triangular mask.
                    diag = j >= 4 * qc
                    off = (j - 4 * qc) * 128 if diag else 0
                    st_ps = stp.tile([128, 512], F32, tag="st", name="st_ps")
                    nc.tensor.matmul(st_ps[:, off:], kT[:, j * 128:(j + 1) * 128],
                                     qT[:, qc * 512 + off:(qc + 1) * 512],
                                     start=True, stop=True)
                    pt = ptp.tile([128, 512], BF16, tag="pt", name="pt")
                    nc.scalar.activation(pt[:, off:], st_ps[:, off:], AF.Exp, scale=SCALE)
                    if diag:
                        nc.vector.tensor_mul(pt[:, off:off + 128],
                                             pt[:, off:off + 128], mask_sb)
                    pts[j] = (pt, off)

                def accum(j):
                    pt, off = pts.pop(j)
                    nc.tensor.matmul(ot_ps[:, off:], vnat[:, j * 128:(j + 1) * 128],
                                     pt[:, off:],
                                     start=(j == 0), stop=(j == nj - 1))
                    nc.tensor.matmul(s_ps[:, off:], ones_sq, pt[:, off:],
                                     start=(j == 0), stop=(j == nj - 1))

                for j in range(nj + KD):
                    if j < nj:
                        score(j)
                    if j >= KD:
                        accum(j - KD)
                rb = rbsb.tile([128, 512], BF16, tag="rbs", name="rb")
                nc.vector.reciprocal(rb, s_ps)
                nc.vector.tensor_mul(OT[hh][:, qc * 512:(qc + 1) * 512], ot_ps, rb)
                if hh + 1 < QH:
                    # next head's in-place rotation trickles out under this
                    # head's attention, one column chunk per qc
                    rope(hh + 1, part=(qc, QC))

    # ---- Phase 4: yT = Wp.T @ OT, streamed out as fp16 partials; PSUM
    # drains alternate ACT/DVE; one batched DMA per 128-row tile ----
    with tc.tile_pool(name="ysb", bufs=4) as ysp, \
         tc.tile_pool(name="yps", bufs=6, space="PSUM") as ypp:
        for nt in (range(32) if upto in ('full', 'p4') else range(0)):
            if nt in wps_pre:
                wps = wps_pre.pop(nt)
            else:
                wps = wpp.tile([128, 512], BF16, tag="wp", name="wp_sb")
                nc.sync.dma_start(wps, wp[nt])
            ysb = ysp.tile([128, S], F16, tag="ysb", name="y_sb")
            for qc in range(QC):
                yps = ypp.tile([128, 512], F32, tag="y", name="y_ps")
                for kb in range(QH):
                    nc.tensor.matmul(yps, wps[:, kb * 128:(kb + 1) * 128],
                                     OT[kb][:, qc * 512:(qc + 1) * 512],
                                     start=(kb == 0), stop=(kb == QH - 1))
                if qc % 2 == 0:
                    nc.scalar.copy(ysb[:, qc * 512:(qc + 1) * 512], yps)
                else:
                    nc.vector.tensor_copy(ysb[:, qc * 512:(qc + 1) * 512], yps)
            nc.scalar.dma_start(yt[nt], ysb)

    for p in (wpp, otp, vpool, ropep, qkvp, cpool):
        p.release()


def build_program(reps=1, upto='full'):
    nc = bass.Bass("TRN2", target_bir_lowering=False, debug=False)
    ht = nc.dram_tensor("ht", [MC, NG, 128, 8, 512], BF16, kind="ExternalInput").ap()
    wq = nc.dram_tensor("wq", [NT, 128, 4096], BF16, kind="ExternalInput").ap()
    bq = nc.dram_tensor("bq", [128, NT], F32, kind="ExternalInput").ap()
    wp = nc.dram_tensor("wp", [32, 128, 512], BF16, kind="ExternalInput").ap()
    cost = nc.dram_tensor("cost", [128, S], BF16, kind="ExternalInput").ap()
    sint = nc.dram_tensor("sint", [128, S], BF16, kind="ExternalInput").ap()
    mask = nc.dram_tensor("mask", [128, 128], BF16, kind="ExternalInput").ap()
    ident = nc.dram_tensor("ident", [128, 128], BF16, kind="ExternalInput").ap()
    yt = nc.dram_tensor("yt", [32, 128, S], F16, kind="ExternalOutput").ap()
    aps = (ht, wq, bq, wp, cost, sint, mask, ident, yt)

    ctx_lp = nc.allow_low_precision(reason="compensated fp8 / bf16 matmuls; fp32 PSUM accumulation")
    ctx_lp.__enter__()
    with PatchedTileContext(nc) as tc:
        for _rep in range(reps):
            _emit_body(nc, tc, aps, upto=upto)
    ctx_lp.__exit__(None, None, None)
    _split_multi_waits(nc)
    return nc


def host_prep(positions, hidden_states, Wqkv, bqkv, Wproj):
    pos = np.asarray(positions).reshape(S)
    h = np.asarray(hidden_states, dtype=np.float32).reshape(S, H)
    Wqkv = np.asarray(Wqkv, dtype=np.float32)
    bqkv = np.asarray(bqkv, dtype=np.float32)
    Wproj = np.asarray(Wproj, dtype=np.float32)

    # hT as [MC, NG, 128, 8, 512] bf16 (8 k-tiles per DMA sub-tile)
    ht_t = np.ascontiguousarray(
        h.reshape(MC, 512, NG, 8, 128).transpose(0, 2, 4, 3, 1)).astype(BF16NP)

    inv_freq = (np.float32(1.0) / (np.float32(ROPE_THETA) **
                (np.arange(HALF, dtype=np.float32) / np.float32(HALF)))).astype(np.float32)
    ang = pos.astype(np.float32)[:, None] * inv_freq[None, :]
    cos = np.cos(ang).astype(np.float32).T     # [64, S]
    sin = np.sin(ang).astype(np.float32).T
    cost = np.ascontiguousarray(np.concatenate([cos, cos], axis=0)).astype(BF16NP)
    sint = np.ascontiguousarray(np.concatenate([sin, sin], axis=0)).astype(BF16NP)

    dk = np.arange(128)[:, None]
    dq = np.arange(128)[None, :]
    mask = np.ascontiguousarray((dq >= dk).astype(np.float32)).astype(BF16NP)
    ident = np.eye(128, dtype=np.float32).astype(BF16NP)

    shared = {"ht": ht_t, "cost": cost, "sint": sint,
              "mask": mask, "ident": ident}
    per_core = []
    for c in range(NCORES):
        Wc = np.concatenate([
            Wqkv[:, 512 * c: 512 * (c + 1)],
            Wqkv[:, H + 128 * c: H + 128 * (c + 1)],
            Wqkv[:, H + 1024 + 128 * c: H + 1024 + 128 * (c + 1)],
        ], axis=1)
        bc = np.concatenate([
            bqkv[512 * c: 512 * (c + 1)],
            bqkv[H + 128 * c: H + 128 * (c + 1)],
            bqkv[H + 1024 + 128 * c: H + 1024 + 128 * (c + 1)],
        ])
        wq_t = np.ascontiguousarray(
            Wc.reshape(KK, 128, NT, 128).transpose(2, 1, 0, 3).reshape(NT, 128, 4096)).astype(BF16NP)
        bq_t = np.ascontiguousarray(bc.reshape(NT, 128).T)
        Wp_c = Wproj[512 * c: 512 * (c + 1), :]
        wp_t = np.ascontiguousarray(
            Wp_c.reshape(QH, 128, 32, 128).transpose(2, 1, 0, 3).reshape(32, 128, 512)).astype(BF16NP)
        per_core.append({"wq": wq_t, "bq": bq_t, "wp": wp_t})
    return shared, per_core


_NC = None


def _get_nc():
    global _NC
    if _NC is None:
        _NC = build_program()
    return _NC


def kernel(positions, hidden_states, Wqkv, bqkv, Wproj, bproj):
    shared, per_core = host_prep(positions, hidden_states, Wqkv, bqkv, Wproj)
    nc = _get_nc()
    in_maps = [dict(shared, **per_core[c]) for c in range(NCORES)]
    res = run_bass_kernel_spmd(nc, in_maps, core_ids=list(range(NCORES)))
    acc = np.zeros((H, S), np.float32)
    for c in range(NCORES):
        acc += res.results[c]["yt"].astype(np.float32).reshape(H, S)
    y = acc.T + np.asarray(bproj, dtype=np.float32)[None, :]
    return y.reshape(1, S, H).astype(np.float32)
